# revision 20
# baseline (speedup 1.0000x reference)
"""Trainium2 Bass kernel for nn_Bottleneck_DCNv3 (8-core SPMD).

Strategy: data-parallel over pixels (2 samples x 4 row-blocks of 16 rows, one
block per NeuronCore; per-core inputs are host-sliced shards, outputs are
host-concatenated -- no collectives).

The DCNv3 deformable bilinear gather + mask blend is reformulated as a banded
matrix multiply per 128-pixel out-tile: blend = M @ window(xpw), where
xpw = cv1_out @ (in_w @ out_w @ diag(bn2_s)) is the output-projected image and
M's 81 nonzeros/row (9x9 bins) are per-pixel "tent" products
K[u,v] = sum_p softmax-mask_p * tent(offy_p - (u-dy_p)) * tent(offx_p - (v-dx_p)),
computed with replicated-column GEMMs + ACT ops, scattered into M in DRAM via
strided (diagonal) DMA descriptors, then transposed back to SBUF with the
hardware DMA-transpose (xbar) path and consumed by TensorE.

All matmul/vector data is bf16 (fp32 PSUM accumulation); BN2 scale is folded
into the projection GEMM and its bias enters as an extra K=1 matmul row, so
stage D is silu via tanh identity: silu(z) = (tanh(z/2)+1)*(z/2).

Validity/borders are handled by a zero ring of width 4 around each shard and
a per-core interior mask that also carries the input_proj bias (extra GEMM
K-row), reproducing grid_sample zero-padding semantics exactly.
"""
import os
import sys
from contextlib import ExitStack

import numpy as np

if '/opt/trn_rl_repo' not in sys.path:
    sys.path.insert(0, '/opt/trn_rl_repo')

import concourse.bass as bass
import concourse.bacc as bacc
import concourse.tile as tile
from concourse import mybir
from concourse._compat import with_exitstack
from concourse.masks import make_identity
from concourse.bass_utils import run_bass_kernel_spmd
import concourse.bass_utils as _bu

AF = mybir.ActivationFunctionType
OP = mybir.AluOpType
FP = mybir.dt.float32
BF = mybir.dt.bfloat16
NPBF = mybir.dt.np(BF)

C = 256
H = W = 64
N = 2
ROWS = 16
YR = 24
XR = 26
WPAD = 72
YF = YR * WPAD          # 1728
XF = XR * WPAD          # 1872
XBUF = 1 + XF + 7       # 1880
PX = ROWS * 64          # 1024
NT = 7
NB = 9
NTILE = PX // 128       # 8
QW = 768                # padded band width per out-tile (6*128)
XPWR = 7 * 144 + QW     # 1776 xpw rows incl. zero pad tail
PXC = 512
LN_EPS = 1e-5

LAST_EXEC_NS = None
LAST_RESULTS = None


# ---------------------------------------------------------------- host prep
def host_consts(inputs):
    """Shared (core-independent) constant tensors."""
    bf = lambda a: np.ascontiguousarray(np.asarray(a, np.float32), ).astype(NPBF)
    f32 = lambda a: np.ascontiguousarray(a, np.float32)
    cons = {}
    w1 = np.asarray(inputs['w1'], np.float32)  # (co, ci, 3, 3)
    w1t = np.zeros((128, 9 * 2 * 256), np.float32)
    for tap in range(9):
        for cic in range(2):
            blk = w1[:, cic * 128:(cic + 1) * 128, tap // 3, tap % 3].T
            w1t[:, (tap * 2 + cic) * 256:(tap * 2 + cic + 1) * 256] = blk
    cons['w1t'] = bf(w1t)

    s1 = inputs['bn1_g'] / np.sqrt(inputs['bn1_v'] + 1e-5)
    cons['bn1s'] = f32(np.stack([s1[:128], s1[128:]], 1))
    b1 = inputs['bn1_b'] - inputs['bn1_m'] * s1
    cons['bn1b'] = f32(np.stack([b1[:128], b1[128:]], 1))

    s2 = np.asarray(inputs['bn2_g'] / np.sqrt(inputs['bn2_v'] + 1e-5), np.float32)
    b2 = np.asarray(inputs['bn2_b'] - inputs['bn2_m'] * s2
                    + s2 * inputs['out_b'], np.float32)
    # W2' = in_w @ out_w @ diag(s2) * 0.5 (0.5 for the tanh-silu identity)
    W2 = (np.asarray(inputs['in_w'], np.float32)
          @ np.asarray(inputs['out_w'], np.float32)) * s2[None, :] * 0.5
    w2c = np.zeros((128, 2 * 256), np.float32)
    for cic in range(2):
        w2c[:, cic * 256:(cic + 1) * 256] = W2[cic * 128:(cic + 1) * 128, :]
    cons['w2c'] = bf(w2c)
    cons['inbw2'] = bf(((np.asarray(inputs['in_b']) @ np.asarray(inputs['out_w']))
                        * s2 * 0.5)[None, :])
    cons['b2row'] = bf((b2 * 0.5)[None, :])

    dw = np.asarray(inputs['dw_w'], np.float32).reshape(C, 9)
    dwd = np.zeros((128, 2 * 9 * 128), np.float32)
    for g in range(2):
        for tap in range(9):
            np.fill_diagonal(dwd[:, (g * 9 + tap) * 128:(g * 9 + tap + 1) * 128],
                             dw[g * 128:(g + 1) * 128, tap])
    cons['dwd'] = bf(dwd)
    cons['dwb'] = f32(np.stack([inputs['dw_b'][:128], inputs['dw_b'][128:]], 1))
    cons['lng'] = f32(np.stack([inputs['ln_g'][:128], inputs['ln_g'][128:]], 1))
    cons['lnb'] = f32(np.stack([inputs['ln_b'][:128], inputs['ln_b'][128:]], 1))

    off_w = np.asarray(inputs['off_w'], np.float32)
    off_b = np.asarray(inputs['off_b'], np.float32)
    mk_w = np.asarray(inputs['mk_w'], np.float32)
    mk_b = np.asarray(inputs['mk_b'], np.float32)
    KS = np.arange(-3, 4, dtype=np.float32)

    wg1m63 = np.zeros((C, 63), np.float32)
    bg1m63 = np.zeros(63, np.float32)
    wg1t63 = np.zeros((C, 63), np.float32)
    bg1t63 = np.zeros(63, np.float32)
    for ky in range(NT):
        for pp in range(9):
            r = ky * 9 + pp
            wg1m63[:, r] = 0.5 * mk_w[:, pp]
            bg1m63[r] = mk_b[pp]
            wg1t63[:, r] = 0.5 * off_w[:, 2 * pp + 1]
            bg1t63[r] = off_b[2 * pp + 1] - KS[ky]
    wg1m = np.concatenate([wg1m63, wg1m63], 1)
    bg1m = np.concatenate([bg1m63, bg1m63])
    wg1t = np.concatenate([wg1t63, wg1t63], 1)
    bg1t = np.concatenate([bg1t63, bg1t63])
    wg1m_t = np.zeros((128, 2 * 126), np.float32)
    wg1t_t = np.zeros((128, 2 * 126), np.float32)
    for cic in range(2):
        wg1m_t[:, cic * 126:(cic + 1) * 126] = wg1m[cic * 128:(cic + 1) * 128]
        wg1t_t[:, cic * 126:(cic + 1) * 126] = wg1t[cic * 128:(cic + 1) * 128]
    cons['wg1m'] = bf(wg1m_t)
    cons['wg1t'] = bf(wg1t_t)
    cons['bg1m'] = f32(np.pad(bg1m, (0, 2))[:, None])
    cons['bg1t'] = f32(np.pad(bg1t, (0, 2))[:, None])

    wg2 = np.zeros((C, 441), np.float32)
    bg2 = np.zeros(441, np.float32)
    for kx in range(NT):
        for j in range(NT):
            for pp in range(9):
                r = kx * 63 + j * 9 + pp
                wg2[:, r] = 0.5 * off_w[:, 2 * pp]
                bg2[r] = off_b[2 * pp] - KS[kx]
    wg2_t = np.zeros((128, 2 * 441), np.float32)
    for cic in range(2):
        wg2_t[:, cic * 441:(cic + 1) * 441] = wg2[cic * 128:(cic + 1) * 128]
    cons['wg2'] = bf(wg2_t)
    bg2p = np.zeros((128, 4), np.float32)
    for ch in range(4):
        c0, c1 = ch * 126, min(ch * 126 + 126, 441)
        bg2p[:c1 - c0, ch] = bg2[c0:c1]
    cons['bg2'] = bg2p

    S = np.zeros((441, 81), np.float32)
    for kx in range(NT):
        for ky in range(NT):
            for pp in range(9):
                dx, dy = pp // 3 - 1, pp % 3 - 1   # reference tap order
                u = dy + (ky - 3) + 4
                v = dx + (kx - 3) + 4
                S[kx * 63 + ky * 9 + pp, u * NB + v] = 1.0
    ssm = np.zeros((128, 4 * 81), np.float32)
    for ch in range(4):
        c0, c1 = ch * 126, min(ch * 126 + 126, 441)
        ssm[:c1 - c0, ch * 81:(ch + 1) * 81] = S[c0:c1]
    cons['ssm'] = bf(ssm)

    cons['onesA'] = bf(np.full((128, 1), 1.0 / C, np.float32))
    cons['ones1'] = bf(np.ones((128, 128), np.float32))
    return cons


def core_inputs(x, n, r0):
    xs = np.zeros((C, XR, WPAD), np.float32)
    lo, hi = r0 - 5, r0 + 21
    clo, chi = max(lo, 0), min(hi, H)
    xs[:, clo - lo:chi - lo, 4:68] = x[n, :, clo:chi, :]
    xsh = np.zeros((C, XBUF), NPBF)
    xsh[:, 1:1 + XF] = xs.reshape(C, XF).astype(NPBF)
    ym = np.zeros((YR, WPAD), np.float32)
    for b in range(YR):
        if 0 <= r0 - 4 + b < H:
            ym[b, 4:68] = 1.0
    ymr = np.broadcast_to(ym.reshape(1, YF).astype(NPBF), (128, YF))
    # pixel-major residual copy of the interior block: [128, 8 tiles, 256]
    xres = np.ascontiguousarray(
        x[n, :, r0:r0 + 16, :].transpose(1, 2, 0).reshape(8, 128, 256)
        .transpose(1, 0, 2).reshape(128, 8 * 256)).astype(NPBF)
    return {'xsh': xsh, 'ymask': np.ascontiguousarray(ymr), 'xres': xres}


IN_SHAPES = {
    'xsh': ((256, XBUF), BF), 'ymask': ((128, YF), BF),
    'xres': ((128, 8 * 256), BF),
    'w1t': ((128, 4608), BF), 'w2c': ((128, 512), BF),
    'inbw2': ((1, 256), BF), 'b2row': ((1, 256), BF),
    'dwd': ((128, 2304), BF), 'dwb': ((128, 2), FP),
    'bn1s': ((128, 2), FP), 'bn1b': ((128, 2), FP),
    'lng': ((128, 2), FP), 'lnb': ((128, 2), FP),
    'wg1m': ((128, 252), BF), 'wg1t': ((128, 252), BF),
    'bg1m': ((128, 1), FP), 'bg1t': ((128, 1), FP),
    'wg2': ((128, 882), BF), 'bg2': ((128, 4), FP), 'ssm': ((128, 324), BF),
    'onesA': ((128, 1), BF), 'ones1': ((128, 128), BF),
}


# ---------------------------------------------------------------- kernel IR
@with_exitstack
def dcn_kernel(ctx: ExitStack, tc: tile.TileContext, outs, ins):
    nc = tc.nc
    out_dram = outs['out']

    cpool = ctx.enter_context(tc.tile_pool(name="consts", bufs=1))
    wpool = ctx.enter_context(tc.tile_pool(name="work", bufs=1))
    spool = ctx.enter_context(tc.tile_pool(name="small", bufs=2))
    ps_mm = ctx.enter_context(tc.tile_pool(name="psmm", bufs=4, space="PSUM"))
    ps_stat = ctx.enter_context(tc.tile_pool(name="psstat", bufs=2, space="PSUM"))

    def cload(name, shape, dt=BF, eng=None):
        t = cpool.tile(shape, dt, name=name, tag=name)
        (eng or nc.gpsimd).dma_start(t[:], ins[name][:, :])
        return t

    w1t = cload('w1t', [128, 4608], eng=nc.sync)
    x2 = []
    for g in range(2):
        t = wpool.tile([128, XBUF], BF, name=f'x2_{g}', tag=f'x2_{g}')
        eng = nc.sync if g == 0 else nc.scalar
        eng.dma_start(t[:], ins['xsh'][g * 128:(g + 1) * 128, :])
        x2.append(t)
    bn1s = cload('bn1s', [128, 2], FP, eng=nc.scalar)
    bn1b = cload('bn1b', [128, 2], FP, eng=nc.scalar)
    ymb = cload('ymask', [128, YF], eng=nc.scalar)
    dwd = cload('dwd', [128, 2304], eng=nc.sync)
    xres = cload('xres', [128, 8 * 256], eng=nc.sync)
    w2c = cload('w2c', [128, 512])
    inbw2 = cload('inbw2', [1, 256])
    b2row = cload('b2row', [1, 256])
    dwb = cload('dwb', [128, 2], FP, eng=nc.scalar)
    lng = cload('lng', [128, 2], FP, eng=nc.scalar)
    lnb = cload('lnb', [128, 2], FP, eng=nc.scalar)
    wg1m = cload('wg1m', [128, 252])
    wg1t = cload('wg1t', [128, 252])
    bg1m = cload('bg1m', [128, 1], FP)
    bg1t = cload('bg1t', [128, 1], FP)
    wg2 = cload('wg2', [128, 882])
    bg2 = cload('bg2', [128, 4], FP)
    ssm = cload('ssm', [128, 324])
    onesA = cload('onesA', [128, 1])
    ones1 = cload('ones1', [128, 128])
    identF = cpool.tile([128, 128], FP, name='identF', tag='identF')
    make_identity(nc, identF[:])
    epsc = cpool.tile([128, 1], FP, name='epsc', tag='epsc')
    nc.gpsimd.memset(epsc[:], LN_EPS)
    onec = cpool.tile([128, 1], FP, name='onec', tag='onec')
    nc.gpsimd.memset(onec[:], 1.0)

    xpw_pm = nc.dram_tensor('xpw_pm', [XPWR, 256], BF, kind='Internal')
    mdram = nc.dram_tensor('mdram', [NTILE * 128 * QW], BF, kind='Internal')

    zpad = cpool.tile([128, QW], BF, name='zpad', tag='zpad')
    nc.gpsimd.memset(zpad[:], 0.0)
    for t in range(NTILE):
        dstz = bass.AP(tensor=mdram, offset=t * 128 * QW, ap=[[QW, 128], [1, QW]])
        nc.gpsimd.dma_start(out=dstz, in_=zpad[:])
    dstz2 = bass.AP(tensor=xpw_pm, offset=1728 * 256, ap=[[256, 48], [1, 256]])
    nc.gpsimd.dma_start(out=dstz2, in_=zpad[0:48, 0:256])

    # ================= stage A: cv1 + BN/SiLU + ymask =================
    y_sb = [wpool.tile([128, YF], BF, name=f'y_{g}', tag=f'y_{g}') for g in range(2)]
    for g in range(2):
        nc.gpsimd.memset(y_sb[g][:], 0.0)
    for g in range(2):
        accs = [ps_mm.tile([128, PXC], FP, name=f'accA{ch}', tag='mm')
                for ch in range(3)]
        for tap in range(9):
            ky, kx = tap // 3, tap % 3
            for cic in range(2):
                lt = w1t[:, (tap * 2 + cic) * 256 + g * 128:
                         (tap * 2 + cic) * 256 + g * 128 + 128]
                for ch in range(3):
                    base = 1 + (ch * 8 + ky) * WPAD + 3 + kx
                    rhs = x2[cic][:, base:base + 8 * WPAD].rearrange(
                        "p (r w) -> p r w", w=WPAD)[:, :, 0:64]
                    nc.tensor.matmul(
                        accs[ch][:], lhsT=lt, rhs=rhs,
                        start=(tap == 0 and cic == 0),
                        stop=(tap == 8 and cic == 1))
        for ch in range(3):
            ys = spool.tile([128, PXC], BF, name='ys', tag='ys', bufs=2)
            nc.scalar.activation(ys[:], accs[ch][:], AF.Silu,
                                 bias=bn1b[:, g:g + 1], scale=bn1s[:, g:g + 1])
            ydst = y_sb[g][:, ch * 8 * WPAD:ch * 8 * WPAD + 8 * WPAD].rearrange(
                "p (r w) -> p r w", w=WPAD)[:, :, 4:68]
            ymsk = ymb[:, ch * 8 * WPAD:ch * 8 * WPAD + 8 * WPAD].rearrange(
                "p (r w) -> p r w", w=WPAD)[:, :, 4:68]
            ysr = ys[:].rearrange("p (r w) -> p r w", w=64)
            nc.vector.tensor_tensor(ydst, ysr, ymsk, op=OP.mult)

    # ================= stage C1: dw conv (PE) ====================
    x1 = [wpool.tile([128, PX], BF, name=f'x1_{g}', tag=f'x1_{g}') for g in range(2)]
    for g in range(2):
        yr = y_sb[g][:].rearrange("p (r w) -> p r w", w=WPAD)
        x1ps = [ps_mm.tile([128, PXC], FP, name=f'x1p{hc}', tag='mm')
                for hc in range(2)]
        for tap in range(9):
            ky, kx = tap // 3, tap % 3
            for hc in range(2):
                srcap = yr[:, 3 + ky + hc * 8:3 + ky + hc * 8 + 8,
                           3 + kx:3 + kx + 64]
                nc.tensor.matmul(
                    x1ps[hc][:],
                    lhsT=dwd[:, (g * 9 + tap) * 128:(g * 9 + tap + 1) * 128],
                    rhs=srcap, start=(tap == 0), stop=(tap == 8))
        for hc in range(2):
            nc.scalar.activation(x1[g][:, hc * PXC:(hc + 1) * PXC], x1ps[hc][:],
                                 AF.Identity, bias=dwb[:, g:g + 1], scale=1.0)

    # ================= stage C2: LN stats + rstd (all px) ============
    sq = [wpool.tile([128, PX], BF, name=f'sq_{g}', tag=f'sq_{g}') for g in range(2)]
    for g in range(2):
        nc.vector.tensor_tensor(sq[g][:], x1[g][:], x1[g][:], op=OP.mult)

    mu_sb = spool.tile([1, PX], BF, name='mu_sb', tag='mu_sb', bufs=1)
    rs_bf = spool.tile([1, PX], BF, name='rs_bf', tag='rs_bf', bufs=1)
    for pc in range(PX // PXC):
        p0 = pc * PXC
        mu = ps_stat.tile([1, PXC], FP, name='mu', tag='stat')
        for g in range(2):
            nc.tensor.matmul(mu[:], lhsT=onesA[:, :], rhs=x1[g][:, p0:p0 + PXC],
                             start=(g == 0), stop=(g == 1))
        sqm = ps_stat.tile([1, PXC], FP, name='sqm', tag='stat')
        for g in range(2):
            nc.tensor.matmul(sqm[:], lhsT=onesA[:, :], rhs=sq[g][:, p0:p0 + PXC],
                             start=(g == 0), stop=(g == 1))
        nc.scalar.copy(mu_sb[0:1, p0:p0 + PXC], mu[:])
        mu2 = spool.tile([1, PXC], FP, name='mu2', tag='mu2')
        nc.vector.tensor_tensor(mu2[:], mu_sb[0:1, p0:p0 + PXC],
                                mu_sb[0:1, p0:p0 + PXC], op=OP.mult)
        var = spool.tile([1, PXC], FP, name='var', tag='var')
        nc.vector.tensor_tensor(var[:], sqm[:], mu2[:], op=OP.subtract)
        sd = spool.tile([1, PXC], FP, name='sd', tag='sd')
        nc.scalar.activation(sd[:], var[:], AF.Sqrt, bias=epsc[0:1, :], scale=1.0)
        rs = spool.tile([1, PXC], FP, name='rs', tag='rs')
        nc.vector.reciprocal_approx_fast(out=rs[:], in_=sd[:])
        nc.vector.tensor_copy(rs_bf[0:1, p0:p0 + PXC], rs[:])

    # ================= stage C3: normalize + GELU ====================
    x1n = [wpool.tile([128, PX], BF, name=f'x1n_{g}', tag=f'x1n_{g}')
           for g in range(2)]
    for pc in range(PX // PXC):
        p0 = pc * PXC
        mub = ps_mm.tile([128, PXC], FP, name='mub', tag='mm')
        nc.tensor.matmul(mub[:], lhsT=ones1[0:1, :], rhs=mu_sb[0:1, p0:p0 + PXC],
                         start=True, stop=True)
        rsb = ps_mm.tile([128, PXC], FP, name='rsb', tag='mm')
        nc.tensor.matmul(rsb[:], lhsT=ones1[0:1, :], rhs=rs_bf[0:1, p0:p0 + PXC],
                         start=True, stop=True)
        for g in range(2):
            t1 = spool.tile([128, PXC], BF, name='t1', tag='gtmp', bufs=2)
            nc.vector.tensor_tensor(t1[:], x1[g][:, p0:p0 + PXC], mub[:],
                                    op=OP.subtract)
            t2 = spool.tile([128, PXC], BF, name='t2', tag='gtmp', bufs=2)
            nc.vector.tensor_tensor(t2[:], t1[:], rsb[:], op=OP.mult)
            tg = spool.tile([128, PXC], BF, name='tg', tag='tg', bufs=2)
            nc.scalar.activation(tg[:], t2[:], AF.Identity,
                                 bias=lnb[:, g:g + 1], scale=lng[:, g:g + 1])
            u2 = spool.tile([128, PXC], BF, name='u2', tag='gtmp', bufs=2)
            nc.scalar.activation(u2[:], tg[:], AF.Square, bias=0.0,
                                 scale=0.21145944)
            a3 = spool.tile([128, PXC], BF, name='a3', tag='gtmp', bufs=2)
            nc.vector.scalar_tensor_tensor(out=a3[:], in0=u2[:],
                                           scalar=onec[:, :], in1=tg[:],
                                           op0=OP.add, op1=OP.mult)
            th = spool.tile([128, PXC], BF, name='th', tag='gtmp', bufs=2)
            nc.scalar.activation(th[:], a3[:], AF.Tanh, bias=0.0,
                                 scale=0.7978845608028654)
            nc.vector.scalar_tensor_tensor(out=x1n[g][:, p0:p0 + PXC],
                                           in0=th[:], scalar=onec[:, :],
                                           in1=tg[:], op0=OP.add, op1=OP.mult)

    # ====== stage B: xpw (pixel-major) = y.T@W2' + ymask.T(x)inbW2' ==
    stg = wpool.tile([128, 14 * 256], BF, name='stg', tag='stg')
    for b in range(14):
        p0 = b * 128
        w = min(128, YF - p0)
        zp = ps_mm.tile([128, 256], FP, name='zpB', tag='mm')
        for cic in range(2):
            nc.tensor.matmul(zp[0:w, :],
                             lhsT=y_sb[cic][:, p0:p0 + w],
                             rhs=w2c[:, cic * 256:(cic + 1) * 256],
                             start=(cic == 0), stop=False)
        nc.tensor.matmul(zp[0:w, :], lhsT=ymb[0:1, p0:p0 + w],
                         rhs=inbw2[0:1, :], start=False, stop=True)
        nc.vector.tensor_copy(stg[0:w, b * 256:(b + 1) * 256], zp[0:w, :])
    s3 = stg[:].rearrange("p (b c) -> p b c", c=256)
    dstB = bass.AP(tensor=xpw_pm, offset=0,
                   ap=[[256, 128], [128 * 256, 13], [1, 256]])
    nc.sync.dma_start(out=dstB, in_=s3[:, 0:13, :])
    dstB2 = bass.AP(tensor=xpw_pm, offset=13 * 128 * 256,
                    ap=[[256, 64], [1, 256]])
    nc.sync.dma_start(out=dstB2, in_=stg[0:64, 13 * 256:14 * 256])

    # preload all 8 blend windows (only depend on xpw_pm)
    win_all = wpool.tile([128, NTILE * 6 * 256], BF, name='win_all',
                         tag='win_all')
    for t in range(NTILE):
        wsrc = bass.AP(tensor=xpw_pm, offset=t * 144 * 256,
                       ap=[[256, 128], [128 * 256, 6], [1, 256]])
        weng = nc.sync if t % 2 == 0 else nc.scalar
        wdst = win_all[:, t * 1536:(t + 1) * 1536].rearrange(
            "p (c j) -> p c j", j=256)
        weng.dma_start(out=wdst, in_=wsrc)

    # ================= stage C4/D: tents + K + scatter + blend =======
    out_sb = wpool.tile([128, NTILE * 256], BF, name='out_sb', tag='out_sb')

    def emit_blend(t):
        mt = spool.tile([128, 6 * 128], BF, name='mt', tag='mt', bufs=3)
        msrc = bass.AP(tensor=mdram, offset=t * 128 * QW,
                       ap=[[QW, 128], [1, QW]])
        (nc.sync if t % 2 == 0 else nc.scalar).dma_start_transpose(
            mt[:].rearrange("p (c j) -> p c j", j=128), msrc)
        zpre = ps_mm.tile([128, 256], FP, name='zpre', tag='mm')
        for qc in range(6):
            nc.tensor.matmul(
                zpre[:], lhsT=mt[:, qc * 128:(qc + 1) * 128],
                rhs=win_all[:, t * 1536 + qc * 256:t * 1536 + (qc + 1) * 256],
                start=(qc == 0), stop=False)
        nc.tensor.matmul(zpre[:], lhsT=ones1[0:1, :], rhs=b2row[0:1, :],
                         start=False, stop=True)
        thd = spool.tile([128, 256], BF, name='thd', tag='thd', bufs=2)
        nc.scalar.activation(thd[:], zpre[:], AF.Tanh, bias=0.0, scale=1.0)
        outp = spool.tile([128, 256], BF, name='outp', tag='outp', bufs=2)
        nc.vector.scalar_tensor_tensor(out=outp[:], in0=thd[:],
                                       scalar=onec[:, :], in1=zpre[:],
                                       op0=OP.add, op1=OP.mult)
        nc.vector.tensor_tensor(out_sb[:, t * 256:(t + 1) * 256], outp[:],
                                xres[:, t * 256:(t + 1) * 256], op=OP.add)

    kn_sb = wpool.tile([81, PX], FP, name='kn', tag='kn')
    kt_sb = wpool.tile([128, NTILE * 81], BF, name='kt', tag='kt')
    rden_sb = spool.tile([128, NTILE], FP, name='rden', tag='rden', bufs=1)
    for pc in range(PX // PXC):
        p0 = pc * PXC
        g1m = ps_mm.tile([126, PXC], FP, name='g1m', tag='mm')
        for cic in range(2):
            nc.tensor.matmul(g1m[:], lhsT=wg1m[:, cic * 126:(cic + 1) * 126],
                             rhs=x1n[cic][:, p0:p0 + PXC],
                             start=(cic == 0), stop=(cic == 1))
        g1t = ps_mm.tile([126, PXC], FP, name='g1t', tag='mm')
        for cic in range(2):
            nc.tensor.matmul(g1t[:], lhsT=wg1t[:, cic * 126:(cic + 1) * 126],
                             rhs=x1n[cic][:, p0:p0 + PXC],
                             start=(cic == 0), stop=(cic == 1))
        m_sb = spool.tile([126, PXC], BF, name='m_sb', tag='m_sb')
        nc.scalar.activation(m_sb[:], g1m[:], AF.Exp, bias=bg1m[0:126, :],
                             scale=1.0)
        tyab = spool.tile([126, PXC], BF, name='tyab', tag='ttmp', bufs=3)
        nc.scalar.activation(tyab[:], g1t[:], AF.Abs, bias=bg1t[0:126, :],
                             scale=1.0)
        ty = spool.tile([126, PXC], BF, name='ty', tag='ttmp', bufs=3)
        nc.scalar.activation(ty[:], tyab[:], AF.Relu, bias=onec[0:126, :],
                             scale=-1.0)
        A = spool.tile([126, PXC], BF, name='A', tag='A')
        nc.vector.tensor_tensor(A[:], m_sb[:], ty[:], op=OP.mult)

        kps = ps_mm.tile([81, PXC], FP, name='kps', tag='kps', bufs=1)
        for chn in range(4):
            r0c, r1c = chn * 126, min(chn * 126 + 126, 441)
            rows = r1c - r0c
            g2 = ps_mm.tile([126, PXC], FP, name='g2', tag='mm')
            for cic in range(2):
                nc.tensor.matmul(g2[0:rows, :],
                                 lhsT=wg2[:, cic * 441 + r0c:cic * 441 + r1c],
                                 rhs=x1n[cic][:, p0:p0 + PXC],
                                 start=(cic == 0), stop=(cic == 1))
            txab = spool.tile([126, PXC], BF, name='txab', tag='ttmp', bufs=3)
            nc.scalar.activation(txab[0:rows, :], g2[0:rows, :], AF.Abs,
                                 bias=bg2[0:rows, chn:chn + 1], scale=1.0)
            tx = spool.tile([126, PXC], BF, name='tx', tag='ttmp', bufs=3)
            nc.scalar.activation(tx[0:rows, :], txab[0:rows, :], AF.Relu,
                                 bias=onec[0:rows, :], scale=-1.0)
            P = spool.tile([126, PXC], BF, name='P', tag='ttmp', bufs=3)
            nc.vector.tensor_tensor(P[0:rows, :], A[0:rows, :], tx[0:rows, :],
                                    op=OP.mult)
            nc.tensor.matmul(kps[:], lhsT=ssm[0:rows, chn * 81:(chn + 1) * 81],
                             rhs=P[0:rows, :], start=(chn == 0), stop=(chn == 3))
        den = ps_stat.tile([1, PXC], FP, name='den', tag='stat')
        nc.tensor.matmul(den[:], lhsT=ones1[0:9, 0:1], rhs=m_sb[0:9, :],
                         start=True, stop=True)
        dsb = spool.tile([1, PXC], FP, name='dsb', tag='dsb')
        nc.scalar.copy(dsb[:], den[:])
        nc.scalar.copy(kn_sb[:, p0:p0 + PXC], kps[:])

        tdps = ps_stat.tile([128, 4], FP, name='tdps', tag='td', bufs=1)
        for ti in range(4):
            nc.tensor.transpose(tdps[:, ti:ti + 1],
                                in_=dsb[0:1, ti * 128:(ti + 1) * 128],
                                identity=identF[0:1, 0:1])
        td_sb = spool.tile([128, 4], FP, name='td_sb', tag='td_sb', bufs=2)
        nc.scalar.copy(td_sb[:], tdps[:])
        nc.vector.reciprocal_approx_fast(out=rden_sb[:, pc * 4:(pc + 1) * 4],
                                         in_=td_sb[:])
        for t in range(pc * 4, pc * 4 + 4):
            tpk = ps_mm.tile([128, 128], FP, name='tpk', tag='mm')
            nc.tensor.transpose(tpk[:, 0:81],
                                in_=kn_sb[0:81, t * 128:(t + 1) * 128],
                                identity=identF[0:81, 0:81])
            nc.vector.tensor_scalar(out=kt_sb[:, t * 81:(t + 1) * 81],
                                    in0=tpk[:, 0:81],
                                    scalar1=rden_sb[:, t:t + 1], scalar2=None,
                                    op0=OP.mult)
        for t in range(pc * 4, pc * 4 + 4):
            for half in range(2):
                ssrc = kt_sb[half * 64:half * 64 + 64,
                             t * 81:(t + 1) * 81].rearrange(
                    "p (u v) -> p u v", v=9)
                off = t * 128 * QW + half * (64 * QW + WPAD)
                dst = bass.AP(tensor=mdram, offset=off,
                              ap=[[QW + 1, 64], [WPAD, 9], [1, 9]])
                eng = nc.sync if half == 0 else nc.scalar
                eng.dma_start(out=dst, in_=ssrc)
        for t in range(pc * 4, pc * 4 + 4):
            emit_blend(t)
        dstO = bass.AP(tensor=out_dram, offset=pc * 4 * 128 * 256,
                       ap=[[256, 128], [128 * 256, 4], [1, 256]])
        nc.sync.dma_start(
            out=dstO,
            in_=out_sb[:, pc * 4 * 256:(pc + 1) * 4 * 256].rearrange(
                "p (t c) -> p t c", c=256))



# ---------------------------------------------------------------- driver
_CACHED_NC = None


def _build_nc():
    global _CACHED_NC
    if _CACHED_NC is not None:
        return _CACHED_NC
    nc = bacc.Bacc("TRN2", target_bir_lowering=False, debug=False, num_devices=8)
    ins = {}
    for name, (shape, dt) in IN_SHAPES.items():
        ins[name] = nc.dram_tensor(name, list(shape), dt, kind='ExternalInput').ap()
    out_t = nc.dram_tensor('out', [PX, 256], BF, kind='ExternalOutput')
    with nc.allow_low_precision(reason="bf16 matmul/vector pipeline"):
        with tile.TileContext(nc) as tc:
            dcn_kernel(tc, {'out': out_t}, ins)
    nc.compile()
    _CACHED_NC = nc
    return nc


def kernel(**inputs):
    global LAST_EXEC_NS
    inputs = {k: np.asarray(v) for k, v in inputs.items()}
    x = np.asarray(inputs['x'], np.float32)
    cons = host_consts(inputs)
    in_maps = []
    shards = []
    for core in range(8):
        n, r0 = core // 4, (core % 4) * 16
        shards.append((n, r0))
        im = dict(cons)
        im.update(core_inputs(x, n, r0))
        in_maps.append(im)

    nc = _build_nc()
    res = run_bass_kernel_spmd(nc, in_maps, core_ids=list(range(8)))
    global LAST_RESULTS
    LAST_RESULTS = res
    LAST_EXEC_NS = res.exec_time_ns

    out = np.zeros((N, C, H, W), np.float32)
    for core, (n, r0) in enumerate(shards):
        o = np.asarray(res.results[core]['out'], dtype=np.float32)
        out[n, :, r0:r0 + 16, :] = o.reshape(ROWS, 64, C).transpose(2, 0, 1)
    return out
